# revision 1
# baseline (speedup 1.0000x reference)
"""Trainium2 Bass kernel for nn_CompactControlAttention.

The module's attention is degenerate: softmax over a size-1 axis is exactly
1.0, so queries/keys (Wq, bq, Wk, bk) never affect the output:

    out[b, s, :] = sequence[b, s, :] + p[b, :]
    p = (sum_c controls[c]) @ Wv.T @ Wo.T + C * (bv @ Wo.T + bo)

Sharding (8 cores, no collectives): tensor-parallel over the OUTPUT feature
dim e. Core k computes out[:, :, 256k:256(k+1)], which needs full Wv
(replicated), a 256-column slice of Wo, and the matching slice of
sequence/bo. Weight matrices are shipped pre-transposed ([in, out] layout)
so the contraction dim lands on SBUF partitions without on-device
transposes of the big weights.

Per-core device program:
  cs = sum_c controls[c]                 (DVE tree + last fold fused below)
  csT = cs.T                             (16 matmuls against stacked identity)
  v = cs @ Wv.T + C*bv                   (PSUM accum over 16 K-tiles)
  vT = v.T                               (16 PE transposes)
  p = v @ WoT_k + bo                     (col-tiled into a [128,128] PSUM tile)
  out = seq_k + broadcast_s(p)           (free-dim step-0 broadcast on DVE)

MM_DT selects TensorEngine precision for the two big GEMMs:
  bf16: weights rounded to bf16 host-side, activations cast on-device,
    fp32 PSUM accumulation. ~1e-3 rel err, fastest (half DMA + 1 cyc/row).
  f32r: tf32-like 2xbf16 decomposition, ~1e-4 rel err, ~2 cyc/row.
  f32: exact fp32, ~4e-7 rel err, 4 cyc/row.
"""

import numpy as np
import ml_dtypes

import concourse.bass as bass
import concourse.mybir as mybir
import concourse.tile as tile
from concourse import bacc
from concourse.bass_utils import run_bass_kernel_spmd
from concourse.masks import make_identity

N_CORES = 8
D = 2048
B = 64
S = 32
C = 8
EK = D // N_CORES  # 256: output-feature slice per core
F32 = mybir.dt.float32
F32R = mybir.dt.float32r
BF16 = mybir.dt.bfloat16

MM_DT = "bf16"  # "bf16" | "f32r" | "f32"

_CACHE = {}


def _build_nc(mm_dt):
    w_dt = BF16 if mm_dt == "bf16" else F32  # DRAM dtype of shipped weights
    nc = bacc.Bacc("TRN2", target_bir_lowering=False, debug=False, num_devices=N_CORES)

    seq = nc.dram_tensor("seq", [128, S * 128], F32, kind="ExternalInput")
    ctrl = nc.dram_tensor("ctrl", [C * B, D], F32, kind="ExternalInput")
    wvt = nc.dram_tensor("wvt", [D, D], w_dt, kind="ExternalInput")  # Wv.T [f, d]
    wot = nc.dram_tensor("wot", [D, EK], w_dt, kind="ExternalInput")  # Wo.T[:, e_k]
    bv = nc.dram_tensor("bv", [D], F32, kind="ExternalInput")
    bo = nc.dram_tensor("bo", [EK], F32, kind="ExternalInput")
    out = nc.dram_tensor("out", [128, S * 128], F32, kind="ExternalOutput")

    with tile.TileContext(nc) as tc:
        _body(tc, seq, ctrl, wvt, wot, bv, bo, out, mm_dt)
    nc.compile()
    return nc


def _body(tc, seq, ctrl, wvt, wot, bv, bo, out, mm_dt):
    from contextlib import ExitStack

    mdt = {"bf16": BF16, "f32r": F32R, "f32": F32}[mm_dt]

    ctx = ExitStack()
    nc = tc.nc
    P = 128

    consts = ctx.enter_context(tc.tile_pool(name="consts", bufs=1))
    sbuf = ctx.enter_context(tc.tile_pool(name="sbuf", bufs=1))
    wpool = ctx.enter_context(tc.tile_pool(name="wv", bufs=3))
    psum_t = ctx.enter_context(tc.tile_pool(name="psum_t", bufs=2, space="PSUM"))
    psum_v = ctx.enter_context(tc.tile_pool(name="psum_v", bufs=1, space="PSUM"))
    psum_p = ctx.enter_context(tc.tile_pool(name="psum_p", bufs=1, space="PSUM"))

    # --- controls first: split across both DMA queues (SWDGE + HWDGE) so
    # the cs chain -- the kernel's critical-path prefix -- is not starved
    # behind weight traffic (both queue FIFOs process it first).
    ctrl_sb = sbuf.tile([P, 4 * D], F32)
    nc.gpsimd.dma_start(
        out=ctrl_sb[:, 0 : 2 * D].rearrange("p (g d) -> p g d", d=D),
        in_=ctrl[0 : 2 * P, :].rearrange("(g p) d -> p g d", p=P),
    )
    nc.sync.dma_start(
        out=ctrl_sb[:, 2 * D : 4 * D].rearrange("p (g d) -> p g d", d=D),
        in_=ctrl[2 * P : 4 * P, :].rearrange("(g p) d -> p g d", p=P),
    )

    # --- constants -------------------------------------------------------
    ident = consts.tile([P, P], F32)
    make_identity(nc, ident[:])
    # sel = two stacked 64x64 identities: a matmul against sel folds the
    # last c-parity pair while transposing.
    sel = consts.tile([P, B], F32)
    nc.gpsimd.dma_start(out=sel[0:B, :], in_=ident[0:B, 0:B])
    nc.gpsimd.dma_start(out=sel[B : 2 * B, :], in_=ident[0:B, 0:B])
    ones8_f = consts.tile([1, B], F32)
    nc.vector.memset(ones8_f[:], float(C))
    ones1_f = consts.tile([1, B], F32)
    nc.vector.memset(ones1_f[:], 1.0)
    ones8 = consts.tile([1, B], mdt)  # value C: bias-augment row for MM1
    nc.vector.tensor_copy(ones8[:], ones8_f[:])
    ones1 = consts.tile([1, B], mdt)  # value 1: bias-augment row for MM2
    nc.vector.tensor_copy(ones1[:], ones1_f[:])

    ident_t = ident if mdt == F32 else consts.tile([P, P], mdt, name="ident_t")
    if mdt != F32:
        nc.vector.tensor_copy(ident_t[:], ident[:])

    # --- fold controls over C --------------------------------------------
    acc = sbuf.tile([P, D], F32)
    nc.vector.tensor_add(acc[:], ctrl_sb[:, 0:D], ctrl_sb[:, D : 2 * D])
    nc.vector.tensor_add(acc[:], acc[:], ctrl_sb[:, 2 * D : 3 * D])
    nc.vector.tensor_add(acc[:], acc[:], ctrl_sb[:, 3 * D : 4 * D])

    # --- csT: fold last c-pair + transpose in one matmul per f-block -----
    cst = sbuf.tile([P, 16 * B], mdt)  # block j at cols [64j, 64j+64)
    for j in range(16):
        pt = psum_t.tile([P, B], F32, tag="pt")
        nc.tensor.matmul(
            pt[:], acc[:, j * P : (j + 1) * P], sel[:], start=True, stop=True
        )
        nc.vector.tensor_copy(cst[:, j * B : (j + 1) * B], pt[:])

    # --- MM1: v = cs @ Wv.T + C*bv  (v in 4 PSUM banks of [64, 512]) -----
    # f32r needs a rounding producer: SWDGE cast-DMA. bf16/f32 ship native.
    wv_dma = nc.gpsimd.dma_start if mm_dt == "f32r" else nc.sync.dma_start
    pv = [psum_v.tile([B, 512], F32, tag=f"pv{c}", name=f"pv{c}") for c in range(4)]
    for jj in range(8):  # stream Wv.T in chunks of two 128-row tiles
        wv_sb = wpool.tile([P, 2 * D], mdt)
        wv_dma(
            out=wv_sb[:].rearrange("p (g d) -> p g d", d=D),
            in_=wvt[jj * 256 : (jj + 1) * 256, :].rearrange("(g p) d -> p g d", p=P),
        )
        for g in range(2):
            j = 2 * jj + g
            for c in range(4):
                nc.tensor.matmul(
                    pv[c][:],
                    cst[:, j * B : (j + 1) * B],
                    wv_sb[:, g * D + c * 512 : g * D + (c + 1) * 512],
                    start=(j == 0),
                    stop=False,
                )
    bv_sb = consts.tile([1, D], mdt)
    nc.gpsimd.dma_start(out=bv_sb[:], in_=bv[None, :])
    for c in range(4):  # bias-augment row: += C * bv
        nc.tensor.matmul(
            pv[c][:],
            ones8[:],
            bv_sb[:, c * 512 : (c + 1) * 512],
            start=False,
            stop=True,
        )
    v = sbuf.tile([B, D], mdt)
    for c in range(4):
        nc.vector.tensor_copy(v[:, c * 512 : (c + 1) * 512], pv[c][:])

    # --- late inputs: issued after the wv stream in each queue FIFO ------
    wo_sb = sbuf.tile([P, 16 * EK], mdt)  # d-tile t at cols [256t, 256t+256)
    wo_dma = nc.gpsimd.dma_start if mm_dt == "f32r" else nc.sync.dma_start
    wo_dma(
        out=wo_sb[:].rearrange("p (t e) -> p t e", e=EK),
        in_=wot.rearrange("(t p) e -> p t e", p=P),
    )
    bo_sb = consts.tile([1, EK], mdt)
    nc.gpsimd.dma_start(out=bo_sb[:], in_=bo[None, :])
    seq_sb = sbuf.tile([P, S * 128], F32)
    nc.sync.dma_start(out=seq_sb[:], in_=seq[:])

    # --- vT: 16 PE transposes --------------------------------------------
    vt = sbuf.tile([P, 16 * B], mdt)
    for t in range(16):
        pt = psum_t.tile([P, B], mdt, name="ptv", tag="pt")
        nc.tensor.transpose(pt[:], v[:, t * P : (t + 1) * P], ident_t[0:B, 0:B])
        nc.vector.tensor_copy(vt[:, t * B : (t + 1) * B], pt[:])

    # --- MM2: p = v @ WoT_k + bo, col-tiled into [128, 128] --------------
    pp = psum_p.tile([P, P], F32, tag="pp")
    for half in range(2):
        o = pp[half * B : (half + 1) * B, :]
        for t in range(16):
            nc.tensor.matmul(
                o,
                vt[:, t * B : (t + 1) * B],
                wo_sb[:, t * EK + half * P : t * EK + (half + 1) * P],
                start=(t == 0),
                stop=False,
            )
        nc.tensor.matmul(
            o, ones1[:], bo_sb[:, half * P : (half + 1) * P], start=False, stop=True
        )
    p_re = sbuf.tile([P, P], F32)
    nc.vector.tensor_copy(p_re[:], pp[:])

    # --- sequence + broadcast(p) -----------------------------------------
    # seq layout (host-prepared): partition p = 64*eh + b, free = (s, e');
    # p broadcasts along the free s-dim (step-0), which DVE supports.
    out_sb = sbuf.tile([P, S * 128], F32)
    nc.vector.tensor_add(
        out_sb[:].rearrange("p (s e) -> p s e", e=P),
        seq_sb[:].rearrange("p (s e) -> p s e", e=P),
        p_re[:, None, :].to_broadcast((P, S, P)),
    )
    nc.sync.dma_start(out=out[:], in_=out_sb[:])
    ctx.close()


def _get_nc(mm_dt=None):
    mm_dt = mm_dt or MM_DT
    if mm_dt not in _CACHE:
        _CACHE[mm_dt] = _build_nc(mm_dt)
    return _CACHE[mm_dt]


def _shard(sequence, controls, Wv, bv, Wo, bo, mm_dt):
    wnp = ml_dtypes.bfloat16 if mm_dt == "bf16" else np.float32
    wvt = np.ascontiguousarray(Wv.T.astype(wnp))
    ctrl = np.ascontiguousarray(controls.reshape(C * B, D))
    in_maps = []
    for k in range(N_CORES):
        ek = slice(k * EK, (k + 1) * EK)
        in_maps.append(
            {
                "seq": np.ascontiguousarray(
                    sequence[:, :, ek]
                    .reshape(B, S, 2, 128)
                    .transpose(2, 0, 1, 3)
                    .reshape(128, S * 128)
                ),
                "ctrl": ctrl,
                "wvt": wvt,
                "wot": np.ascontiguousarray(Wo[ek, :].T.astype(wnp)),
                "bv": np.ascontiguousarray(bv),
                "bo": np.ascontiguousarray(bo[ek]),
            }
        )
    return in_maps


def _run(inputs, trace=False, mm_dt=None):
    mm_dt = mm_dt or MM_DT
    nc = _get_nc(mm_dt)
    in_maps = _shard(
        np.asarray(inputs["sequence"]), np.asarray(inputs["controls"]),
        np.asarray(inputs["Wv"]), np.asarray(inputs["bv"]),
        np.asarray(inputs["Wo"]), np.asarray(inputs["bo"]), mm_dt,
    )
    res = run_bass_kernel_spmd(nc, in_maps, list(range(N_CORES)), trace=trace)
    out = np.empty((B, S, D), dtype=np.float32)
    for k in range(N_CORES):
        out[:, :, k * EK : (k + 1) * EK] = (
            res.results[k]["out"]
            .reshape(2, B, S, 128)
            .transpose(1, 2, 0, 3)
            .reshape(B, S, EK)
        )
    return out, res


def kernel(**inputs):
    out, _ = _run(inputs)
    return out



# revision 3
# speedup vs baseline: 1.9329x; 1.9329x over previous
"""Trainium2 Bass kernel for nn_CompactControlAttention.

The module's attention is degenerate: softmax over a size-1 axis is exactly
1.0, so queries/keys (Wq, bq, Wk, bk) never affect the output:

    out[b, s, :] = sequence[b, s, :] + p[b, :]
    p = (sum_c controls[c]) @ Wv.T @ Wo.T + C * (bv @ Wo.T + bo)

Sharding (8 cores, no collectives): tensor-parallel over the OUTPUT feature
dim e.  Core k computes out[:, :, 256k:256(k+1)], which needs full Wv
(replicated), a 256-column slice of Wo, and the matching slices of
sequence/bo.  Cross-core collectives were measured to pay a ~50-70us
first-collective rendezvous (core launch skew under this runtime), so the
kernel is communication-free and optimized purely as a streaming pipeline.

Pipeline (per core), everything overlapped with the 12 MB input stream:
  phase A: ctrl (bf16) streams in 8 d-chunks; each chunk is folded over C
    (pair-adds into f32) and transposed via a stacked-identity matmul that
    also folds the last c-parity pair -> csT blocks (bf16).
  phase B: Wv.T streams in 4 COLUMN panels (host-pretransposed so each
    panel is contiguous).  Panel c: 16 K-tile matmuls accumulate
    v[:, 512c:512c+512]; then bias, PSUM->SBUF cast, 4 PE transposes and
    8 MM2 matmuls for those v-features run while panel c+1 streams.
  tail: p = pp + C*bo; out = seq + broadcast_s(p) in 4 chunks (ACT engine
    casts seq bf16->f32, DVE adds, writes stream on the scalar queue).

Precision: weights/activations bf16 (fp32 PSUM accumulation), sequence
shipped bf16 (its rounding is ~1e-4 of the output scale).  Measured rel
err ~3e-3 against the fp32 reference.
"""

import numpy as np
import ml_dtypes

import concourse.bass as bass
import concourse.mybir as mybir
import concourse.tile as tile
from concourse import bacc
from concourse.bass_utils import run_bass_kernel_spmd
from concourse.masks import make_identity

N_CORES = 8
D = 2048
B = 64
S = 32
C = 8
EK = D // N_CORES  # 256: per-core output-feature slice
F32 = mybir.dt.float32
BF16 = mybir.dt.bfloat16
P = 128
NPANEL = 4
PW = D // NPANEL  # 512 v-features per panel
NCC = 8
CD = D // NCC  # 256 ctrl d-columns per chunk

_CACHE = {}


def _build_nc():
    nc = bacc.Bacc("TRN2", target_bir_lowering=False, debug=False, num_devices=N_CORES)

    seq = nc.dram_tensor("seq", [P, S * P], BF16, kind="ExternalInput")
    ctrl = nc.dram_tensor("ctrl", [C * B, D], BF16, kind="ExternalInput")
    # Wv.T reorganized host-side into NPANEL contiguous column panels:
    # wvp[c] = Wv.T[:, 512c:512(c+1)]  ->  [NPANEL * D, PW]
    wvp = nc.dram_tensor("wvp", [NPANEL * D, PW], BF16, kind="ExternalInput")
    wot = nc.dram_tensor("wot", [D, EK], BF16, kind="ExternalInput")  # Wo.T[:, ek]
    bv = nc.dram_tensor("bv", [1, D], F32, kind="ExternalInput")
    bo = nc.dram_tensor("bo", [1, EK], F32, kind="ExternalInput")
    out = nc.dram_tensor("out", [P, S * P], F32, kind="ExternalOutput")

    with tile.TileContext(nc) as tc:
        _body(tc, seq, ctrl, wvp, wot, bv, bo, out)
    nc.compile()
    return nc


def _body(tc, seq, ctrl, wvp, wot, bv, bo, out):
    from contextlib import ExitStack

    ctx = ExitStack()
    nc = tc.nc

    consts = ctx.enter_context(tc.tile_pool(name="consts", bufs=1))
    sbuf = ctx.enter_context(tc.tile_pool(name="sbuf", bufs=1))
    wpool = ctx.enter_context(tc.tile_pool(name="wv", bufs=2))
    psum_t = ctx.enter_context(tc.tile_pool(name="psum_t", bufs=2, space="PSUM"))
    psum_v = ctx.enter_context(tc.tile_pool(name="psum_v", bufs=2, space="PSUM"))
    psum_p = ctx.enter_context(tc.tile_pool(name="psum_p", bufs=1, space="PSUM"))

    # --- sync (SP) queue FIFO: ctrl chunks, wot, Wv panels, seq ----------
    ctrl_sb = sbuf.tile([P, NCC * 4 * CD], BF16)  # chunk c: 4 c-groups x CD cols
    for c in range(NCC):
        nc.sync.dma_start(
            out=ctrl_sb[:, c * 4 * CD : (c + 1) * 4 * CD].rearrange(
                "p (t d) -> p t d", d=CD
            ),
            in_=ctrl[:, c * CD : (c + 1) * CD].rearrange("(t p) d -> p t d", p=P),
        )
    wo_sb = sbuf.tile([P, 16 * EK], BF16)  # d-tile t at cols [256t, 256t+256)
    nc.sync.dma_start(
        out=wo_sb[:].rearrange("p (t e) -> p t e", e=EK),
        in_=wot.rearrange("(t p) e -> p t e", p=P),
    )
    wv_sb = []
    for c in range(NPANEL):  # contiguous 2MB panel reads
        w = wpool.tile([P, 16 * PW], BF16, name=f"wvpanel{c % 2}", tag=f"wv{c % 2}")
        nc.sync.dma_start(
            out=w[:].rearrange("p (t f) -> p t f", f=PW),
            in_=wvp[c * D : (c + 1) * D, :].rearrange("(t p) f -> p t f", p=P),
        )
        wv_sb.append(w)
    seq_sb = sbuf.tile([P, S * P], BF16)
    nc.sync.dma_start(out=seq_sb[:], in_=seq[:])

    # --- constants (gpsimd queue / on-engine) ----------------------------
    ident = consts.tile([P, P], F32)
    make_identity(nc, ident[:])
    sel_f = consts.tile([P, B], F32)  # two stacked 64x64 identities
    nc.gpsimd.dma_start(out=sel_f[0:B, :], in_=ident[0:B, 0:B])
    nc.gpsimd.dma_start(out=sel_f[B : 2 * B, :], in_=ident[0:B, 0:B])
    sel = consts.tile([P, B], BF16)
    nc.vector.tensor_copy(sel[:], sel_f[:])
    ident_t = consts.tile([B, B], BF16)
    nc.vector.tensor_copy(ident_t[:], ident[0:B, 0:B])

    onesC_f = consts.tile([1, B], F32)
    nc.vector.memset(onesC_f[:], float(C))
    onesC = consts.tile([1, B], BF16)
    nc.vector.tensor_copy(onesC[:], onesC_f[:])
    ones1_f = consts.tile([1, B], F32)
    nc.vector.memset(ones1_f[:], 1.0)
    ones1 = consts.tile([1, B], BF16)
    nc.vector.tensor_copy(ones1[:], ones1_f[:])

    bv_f = consts.tile([1, D], F32)
    nc.gpsimd.dma_start(out=bv_f[:], in_=bv[:])
    bv_sb = consts.tile([1, D], BF16)
    nc.vector.tensor_copy(bv_sb[:], bv_f[:])
    bo_f = consts.tile([1, EK], F32)
    nc.gpsimd.dma_start(out=bo_f[:], in_=bo[:])
    bo_sb = consts.tile([1, EK], BF16)
    nc.vector.tensor_copy(bo_sb[:], bo_f[:])

    # --- phase A: fold C + transpose, pipelined per ctrl chunk -----------
    a01 = sbuf.tile([P, CD], F32, name="a01")
    a23 = sbuf.tile([P, CD], F32, name="a23")
    acc = sbuf.tile([P, D], F32)
    cst = sbuf.tile([P, 16 * B], BF16)  # csT block j at cols [64j, 64j+64)
    for c in range(NCC):
        base = c * 4 * CD
        nc.vector.tensor_add(
            a01[:], ctrl_sb[:, base : base + CD], ctrl_sb[:, base + CD : base + 2 * CD]
        )
        nc.vector.tensor_add(
            a23[:],
            ctrl_sb[:, base + 2 * CD : base + 3 * CD],
            ctrl_sb[:, base + 3 * CD : base + 4 * CD],
        )
        nc.vector.tensor_add(acc[:, c * CD : (c + 1) * CD], a01[:], a23[:])
        for h in range(CD // P):
            j = c * (CD // P) + h
            pt = psum_t.tile([P, B], F32, tag="pt")
            nc.tensor.matmul(
                pt[:], acc[:, j * P : (j + 1) * P], sel_f[:], start=True, stop=True
            )
            nc.vector.tensor_copy(cst[:, j * B : (j + 1) * B], pt[:])

    # --- phase B: per Wv column panel: MM1, bias, vT, MM2 ----------------
    pp = psum_p.tile([P, P], F32, tag="pp")  # p, partition = 64*eh + b
    for c in range(NPANEL):
        pv = psum_v.tile([B, PW], F32, tag=f"pv{c % 2}", name=f"pv{c % 2}")
        w = wv_sb[c]
        for j in range(16):
            nc.tensor.matmul(
                pv[:],
                cst[:, j * B : (j + 1) * B],
                w[:, j * PW : (j + 1) * PW],
                start=(j == 0),
                stop=False,
            )
        nc.tensor.matmul(  # += C * bv (panel slice)
            pv[:], onesC[:], bv_sb[:, c * PW : (c + 1) * PW], start=False, stop=True
        )
        v = sbuf.tile([B, PW], BF16, name=f"v{c % 2}")
        nc.vector.tensor_copy(v[:], pv[:])
        for h in range(4):
            t = 4 * c + h
            pt = psum_t.tile([P, B], BF16, name="ptv", tag="pt")
            nc.tensor.transpose(pt[:], v[:, h * P : (h + 1) * P], ident_t[:])
            vt = sbuf.tile([P, B], BF16, name=f"vt{t % 4}")
            nc.vector.tensor_copy(vt[:], pt[:])
            for half in range(2):
                nc.tensor.matmul(
                    pp[half * B : (half + 1) * B, :],
                    vt[:],
                    wo_sb[:, t * EK + half * P : t * EK + (half + 1) * P],
                    start=(t == 0),
                    stop=False,
                )
    for half in range(2):  # += 1 * bo
        nc.tensor.matmul(
            pp[half * B : (half + 1) * B, :],
            ones1[:],
            bo_sb[:, half * P : (half + 1) * P],
            start=False,
            stop=(half == 1),
        )
    p_re = sbuf.tile([P, P], F32)
    nc.vector.tensor_copy(p_re[:], pp[:])

    # --- tail: out = seq + broadcast_s(p), 4 chunks ----------------------
    NOUT = 4
    W = S * P // NOUT  # 1024
    out_sb = sbuf.tile([P, S * P], F32)
    for c in range(NOUT):
        sl = slice(c * W, (c + 1) * W)
        nc.scalar.activation(  # ACT: bf16 -> f32 copy
            out_sb[:, sl], seq_sb[:, sl], mybir.ActivationFunctionType.Copy
        )
        nc.vector.tensor_add(
            out_sb[:, sl].rearrange("p (s e) -> p s e", e=P),
            out_sb[:, sl].rearrange("p (s e) -> p s e", e=P),
            p_re[:, None, :].to_broadcast((P, S // NOUT, P)),
        )
        nc.scalar.dma_start(out=out[:, sl], in_=out_sb[:, sl])
    ctx.close()


def _get_nc():
    if "nc" not in _CACHE:
        _CACHE["nc"] = _build_nc()
    return _CACHE["nc"]


def _shard(sequence, controls, Wv, bv, Wo, bo):
    bf = ml_dtypes.bfloat16
    ctrl = np.ascontiguousarray(controls.reshape(C * B, D).astype(bf))
    # Wv.T as NPANEL contiguous column panels: [NPANEL*D, PW]
    wvp = np.ascontiguousarray(
        Wv.T.astype(bf).reshape(D, NPANEL, PW).transpose(1, 0, 2).reshape(NPANEL * D, PW)
    )
    bvr = np.ascontiguousarray(bv[None, :].astype(np.float32))
    in_maps = []
    for k in range(N_CORES):
        sl = slice(k * EK, (k + 1) * EK)
        in_maps.append(
            {
                "seq": np.ascontiguousarray(
                    sequence[:, :, sl]
                    .reshape(B, S, 2, P)
                    .transpose(2, 0, 1, 3)
                    .reshape(P, S * P)
                    .astype(bf)
                ),
                "ctrl": ctrl,
                "wvp": wvp,
                "wot": np.ascontiguousarray(Wo[sl, :].T.astype(bf)),
                "bv": bvr,
                "bo": np.ascontiguousarray(bo[None, sl].astype(np.float32)),
            }
        )
    return in_maps


def _run(inputs, trace=False):
    nc = _get_nc()
    in_maps = _shard(
        np.asarray(inputs["sequence"]), np.asarray(inputs["controls"]),
        np.asarray(inputs["Wv"]), np.asarray(inputs["bv"]),
        np.asarray(inputs["Wo"]), np.asarray(inputs["bo"]),
    )
    res = run_bass_kernel_spmd(nc, in_maps, list(range(N_CORES)), trace=trace)
    out = np.empty((B, S, D), dtype=np.float32)
    for k in range(N_CORES):
        out[:, :, k * EK : (k + 1) * EK] = (
            res.results[k]["out"]
            .reshape(2, B, S, P)
            .transpose(1, 2, 0, 3)
            .reshape(B, S, EK)
        )
    return out, res


def kernel(**inputs):
    out, _ = _run(inputs)
    return out


# revision 4
# speedup vs baseline: 2.1266x; 1.1002x over previous
"""Trainium2 Bass kernel for nn_CompactControlAttention.

The module's attention is degenerate: softmax over a size-1 axis is exactly
1.0, so queries/keys (Wq, bq, Wk, bk) never affect the output:

    out[b, s, :] = sequence[b, s, :] + p[b, :]
    p = (sum_c controls[c]) @ Wv.T @ Wo.T + C * (bv @ Wo.T + bo)

Sharding (8 cores, no collectives): tensor-parallel over the OUTPUT feature
dim e.  Core k computes out[:, :, 256k:256(k+1)], needing full Wv
(replicated), a 256-column slice of Wo, and matching slices of seq/bo.
Cross-core collectives were measured to pay a 50-70us first-collective
rendezvous on this runtime (core launch skew), so the kernel is
communication-free and optimized as a single streaming pipeline:

  - All weight/activation tensors are pre-packed host-side into the exact
    SBUF layout ([128, free], partition-contiguous) so every DMA moves
    contiguous per-partition rows at line rate.
  - ctrl (bf16) streams in 4 sub-chunks; fold over C (pair-adds into f32)
    and the stacked-identity transpose-matmul (folds the last c-parity
    pair) pipeline against the stream -> csT blocks (bf16).
  - A burst of dummy bf16 matmuls right after phase A warms the PE out of
    the HAM 1.2 GHz throttle before the real GEMM work arrives.
  - Wv.T streams as 4 column panels x 4 sub-chunks; MM1 K-tiles fire per
    sub-chunk (PE never idles long enough to re-throttle).  After each
    panel: bias, PSUM->SBUF cast, 4 PE transposes + 8 MM2 matmuls, all
    overlapped with the next panel's stream.
  - tail: p = pp + C*bo; out = seq + broadcast_s(p) in 8 chunks (ACT casts
    seq fp8->f32, DVE adds, output writes stream on the scalar queue).

Precision: bf16 weights/activations with fp32 PSUM accumulation; sequence
ships as fp8e4m3 (adds ~3e-3 of output-scale rounding; it only enters the
final residual add).  Measured rel err ~5e-3 vs the fp32 reference, well
under the 2e-2 gate.
"""

import numpy as np
import ml_dtypes

import concourse.bass as bass
import concourse.mybir as mybir
import concourse.tile as tile
from concourse import bacc
from concourse.bass_utils import run_bass_kernel_spmd
from concourse.masks import make_identity

N_CORES = 8
D = 2048
B = 64
S = 32
C = 8
EK = D // N_CORES  # 256: per-core output-feature slice
F32 = mybir.dt.float32
BF16 = mybir.dt.bfloat16
FP8 = mybir.dt.float8e4
P = 128
NPANEL = 4
PW = D // NPANEL  # 512 v-features per panel
NSUB = 4  # sub-DMAs per panel (4 K-tiles each)
NCC = 4  # ctrl sub-chunks (512 d-cols each)
CD = D // NCC  # 512

_CACHE = {}


def _build_nc():
    nc = bacc.Bacc("TRN2", target_bir_lowering=False, debug=False, num_devices=N_CORES)

    # All inputs pre-packed host-side to SBUF layout [128, free].
    seq = nc.dram_tensor("seq", [P, S * P], FP8, kind="ExternalInput")
    ctrl = nc.dram_tensor("ctrl", [P, 4 * D], BF16, kind="ExternalInput")
    wvp = nc.dram_tensor("wvp", [NPANEL * P, 16 * PW], BF16, kind="ExternalInput")
    wot = nc.dram_tensor("wot", [P, 16 * EK], BF16, kind="ExternalInput")
    bv = nc.dram_tensor("bv", [1, D], F32, kind="ExternalInput")
    bo = nc.dram_tensor("bo", [1, EK], F32, kind="ExternalInput")
    out = nc.dram_tensor("out", [P, S * P], F32, kind="ExternalOutput")

    with tile.TileContext(nc) as tc:
        _body(tc, seq, ctrl, wvp, wot, bv, bo, out)
    nc.compile()
    return nc


def _body(tc, seq, ctrl, wvp, wot, bv, bo, out):
    from contextlib import ExitStack

    ctx = ExitStack()
    nc = tc.nc

    consts = ctx.enter_context(tc.tile_pool(name="consts", bufs=1))
    sbuf = ctx.enter_context(tc.tile_pool(name="sbuf", bufs=1))
    wpool = ctx.enter_context(tc.tile_pool(name="wv", bufs=2))
    psum_t = ctx.enter_context(tc.tile_pool(name="psum_t", bufs=2, space="PSUM"))
    psum_v = ctx.enter_context(tc.tile_pool(name="psum_v", bufs=2, space="PSUM"))
    psum_p = ctx.enter_context(tc.tile_pool(name="psum_p", bufs=1, space="PSUM"))
    psum_d = ctx.enter_context(tc.tile_pool(name="psum_d", bufs=1, space="PSUM"))

    # --- sync (SP) queue FIFO ---------------------------------------------
    # ctrl x4, p0s0, p0s1, wot, p0s2, p0s3, p1s0..p3s3, seq
    # ctrl chunk c holds d-cols [512c, 512c+512) for all 4 row-groups:
    # layout [128, 4 groups x 512]
    ctrl_sb = sbuf.tile([P, 4 * D], BF16)
    for c in range(NCC):
        nc.sync.dma_start(
            out=ctrl_sb[:, c * 4 * CD : (c + 1) * 4 * CD],
            in_=ctrl[:, c * 4 * CD : (c + 1) * 4 * CD],
        )

    wv_sb = []
    wo_sb = sbuf.tile([P, 16 * EK], BF16)

    def panel_subs(c):
        w = wpool.tile([P, 16 * PW], BF16, name=f"wvpanel{c % 2}", tag=f"wv{c % 2}")
        wv_sb.append(w)
        for s in range(NSUB):
            sl = slice(s * 4 * PW, (s + 1) * 4 * PW)
            nc.sync.dma_start(out=w[:, sl], in_=wvp[c * P : (c + 1) * P, sl])
            if c == 0 and s == 1:  # wot mid-panel-0: arrives before MM2-p0
                nc.sync.dma_start(out=wo_sb[:], in_=wot[:])

    for c in range(NPANEL):
        panel_subs(c)
    seq_sb = sbuf.tile([P, S * P], FP8)
    nc.sync.dma_start(out=seq_sb[:], in_=seq[:])

    # --- constants (gpsimd queue / on-engine) ----------------------------
    ident = consts.tile([P, P], F32)
    make_identity(nc, ident[:])
    sel_f = consts.tile([P, B], F32)  # two stacked 64x64 identities
    nc.gpsimd.dma_start(out=sel_f[0:B, :], in_=ident[0:B, 0:B])
    nc.gpsimd.dma_start(out=sel_f[B : 2 * B, :], in_=ident[0:B, 0:B])
    ident_t = consts.tile([B, B], BF16)
    nc.vector.tensor_copy(ident_t[:], ident[0:B, 0:B])

    onesC_f = consts.tile([1, B], F32)
    nc.vector.memset(onesC_f[:], float(C))
    onesC = consts.tile([1, B], BF16)
    nc.vector.tensor_copy(onesC[:], onesC_f[:])
    ones1_f = consts.tile([1, B], F32)
    nc.vector.memset(ones1_f[:], 1.0)
    ones1 = consts.tile([1, B], BF16)
    nc.vector.tensor_copy(ones1[:], ones1_f[:])

    bv_f = consts.tile([1, D], F32)
    nc.gpsimd.dma_start(out=bv_f[:], in_=bv[:])
    bv_sb = consts.tile([1, D], BF16)
    nc.vector.tensor_copy(bv_sb[:], bv_f[:])
    bo_f = consts.tile([1, EK], F32)
    nc.gpsimd.dma_start(out=bo_f[:], in_=bo[:])
    bo_sb = consts.tile([1, EK], BF16)
    nc.vector.tensor_copy(bo_sb[:], bo_f[:])

    # --- phase A: fold C + transpose, pipelined per ctrl chunk -----------
    a01 = sbuf.tile([P, CD], F32, name="a01")
    a23 = sbuf.tile([P, CD], F32, name="a23")
    acc = sbuf.tile([P, D], F32)
    cst = sbuf.tile([P, 16 * B], BF16)  # csT block j at cols [64j, 64j+64)
    for c in range(NCC):
        base = c * 4 * CD
        nc.vector.tensor_add(
            a01[:], ctrl_sb[:, base : base + CD], ctrl_sb[:, base + CD : base + 2 * CD]
        )
        nc.vector.tensor_add(
            a23[:],
            ctrl_sb[:, base + 2 * CD : base + 3 * CD],
            ctrl_sb[:, base + 3 * CD : base + 4 * CD],
        )
        nc.vector.tensor_add(acc[:, c * CD : (c + 1) * CD], a01[:], a23[:])
        for h in range(CD // P):
            j = c * (CD // P) + h
            pt = psum_t.tile([P, B], F32, tag="pt")
            nc.tensor.matmul(
                pt[:], acc[:, j * P : (j + 1) * P], sel_f[:], start=True, stop=True
            )
            nc.vector.tensor_copy(cst[:, j * B : (j + 1) * B], pt[:])
        if c == 0:
            # PE warm-up: ~12 bf16 N=512 matmuls on resident data pull the
            # HAM clock gate to 2.4 GHz before the real GEMM stream starts.
            pd = psum_d.tile([B, PW], F32, tag="pd")
            for i in range(12):
                nc.tensor.matmul(
                    pd[:],
                    cst[:, 0:B],
                    ctrl_sb[:, 0:PW],
                    start=(i == 0),
                    stop=(i == 11),
                )

    # --- phase B: per Wv column panel: MM1, bias, vT, MM2 ----------------
    pp = psum_p.tile([P, P], F32, tag="pp")  # p; partition = 64*eh + b
    for c in range(NPANEL):
        pv = psum_v.tile([B, PW], F32, tag=f"pv{c % 2}", name=f"pv{c % 2}")
        w = wv_sb[c]
        for j in range(16):
            nc.tensor.matmul(
                pv[:],
                cst[:, j * B : (j + 1) * B],
                w[:, j * PW : (j + 1) * PW],
                start=(j == 0),
                stop=False,
            )
        nc.tensor.matmul(  # += C * bv (panel slice)
            pv[:], onesC[:], bv_sb[:, c * PW : (c + 1) * PW], start=False, stop=True
        )
        v = sbuf.tile([B, PW], BF16, name=f"v{c % 2}")
        nc.vector.tensor_copy(v[:], pv[:])
        for h in range(4):
            t = 4 * c + h
            pt = psum_t.tile([P, B], BF16, name="ptv", tag="pt")
            nc.tensor.transpose(pt[:], v[:, h * P : (h + 1) * P], ident_t[:])
            vt = sbuf.tile([P, B], BF16, name=f"vt{t % 4}")
            nc.vector.tensor_copy(vt[:], pt[:])
            for half in range(2):
                nc.tensor.matmul(
                    pp[half * B : (half + 1) * B, :],
                    vt[:],
                    wo_sb[:, t * EK + half * P : t * EK + (half + 1) * P],
                    start=(t == 0),
                    stop=False,
                )
    for half in range(2):  # += 1 * bo
        nc.tensor.matmul(
            pp[half * B : (half + 1) * B, :],
            ones1[:],
            bo_sb[:, half * P : (half + 1) * P],
            start=False,
            stop=(half == 1),
        )
    p_re = sbuf.tile([P, P], F32)
    nc.vector.tensor_copy(p_re[:], pp[:])

    # --- tail: out = seq + broadcast_s(p), 8 chunks ----------------------
    NOUT = 8
    W = S * P // NOUT  # 512 (4 s-steps)
    out_sb = sbuf.tile([P, S * P], F32)
    for c in range(NOUT):
        sl = slice(c * W, (c + 1) * W)
        nc.scalar.activation(  # ACT: fp8 -> f32 copy
            out_sb[:, sl], seq_sb[:, sl], mybir.ActivationFunctionType.Copy
        )
    for c in range(NOUT):
        sl = slice(c * W, (c + 1) * W)
        nc.vector.tensor_add(
            out_sb[:, sl].rearrange("p (s e) -> p s e", e=P),
            out_sb[:, sl].rearrange("p (s e) -> p s e", e=P),
            p_re[:, None, :].to_broadcast((P, S // NOUT, P)),
        )
        nc.scalar.dma_start(out=out[:, sl], in_=out_sb[:, sl])
    ctx.close()


def _get_nc():
    if "nc" not in _CACHE:
        _CACHE["nc"] = _build_nc()
    return _CACHE["nc"]


def _pack_rows(a):
    """[T*128, F] -> [128, T*F]: partition-major SBUF layout, contiguous."""
    T = a.shape[0] // P
    return np.ascontiguousarray(
        a.reshape(T, P, a.shape[1]).transpose(1, 0, 2).reshape(P, T * a.shape[1])
    )


def _shard(sequence, controls, Wv, bv, Wo, bo):
    bf = ml_dtypes.bfloat16
    f8 = ml_dtypes.float8_e4m3
    # ctrl: [512, 2048] -> chunks of 512 d-cols, each [128, 4 x 512]
    cb = controls.reshape(C * B, D).astype(bf)
    ctrl = np.ascontiguousarray(
        cb.reshape(4, P, NCC, CD).transpose(1, 2, 0, 3).reshape(P, 4 * D)
    )
    # Wv.T column panels, each packed to [128, 16*PW]
    wvt = Wv.T.astype(bf)  # [d, f]
    wvp = np.ascontiguousarray(
        np.concatenate(
            [_pack_rows(wvt[:, cc * PW : (cc + 1) * PW]) for cc in range(NPANEL)],
            axis=0,
        )
    )
    bvr = np.ascontiguousarray(bv[None, :].astype(np.float32))
    in_maps = []
    for k in range(N_CORES):
        sl = slice(k * EK, (k + 1) * EK)
        in_maps.append(
            {
                "seq": np.ascontiguousarray(
                    sequence[:, :, sl]
                    .reshape(B, S, 2, P)
                    .transpose(2, 0, 1, 3)
                    .reshape(P, S * P)
                    .astype(f8)
                ),
                "ctrl": ctrl,
                "wvp": wvp,
                "wot": _pack_rows(Wo[sl, :].T.astype(bf)),
                "bv": bvr,
                "bo": np.ascontiguousarray(bo[None, sl].astype(np.float32)),
            }
        )
    return in_maps


def _run(inputs, trace=False):
    nc = _get_nc()
    in_maps = _shard(
        np.asarray(inputs["sequence"]), np.asarray(inputs["controls"]),
        np.asarray(inputs["Wv"]), np.asarray(inputs["bv"]),
        np.asarray(inputs["Wo"]), np.asarray(inputs["bo"]),
    )
    res = run_bass_kernel_spmd(nc, in_maps, list(range(N_CORES)), trace=trace)
    out = np.empty((B, S, D), dtype=np.float32)
    for k in range(N_CORES):
        out[:, :, k * EK : (k + 1) * EK] = (
            res.results[k]["out"]
            .reshape(2, B, S, P)
            .transpose(1, 2, 0, 3)
            .reshape(B, S, EK)
        )
    return out, res


def kernel(**inputs):
    out, _ = _run(inputs)
    return out
